# revision 44
# baseline (speedup 1.0000x reference)
"""Trainium2 Bass kernel for a 6-layer GPT (D=512, H=8, T=1024, B=2, V=50257).

Strategy (8 NeuronCores), v3:
- Token-shard the transformer body: core c owns 256 tokens (cores 0-3 =
  batch 0 chunks 0-3, cores 4-7 = batch 1 chunks 0-3).
- All matmul operands bf16 (PSUM accumulates fp32); residual/LN fp32.
- Host pre-rearranges every weight into partition-major layout so DMA
  descriptors are >=4KB contiguous lines; DMAs are spread over the three
  trigger queues (sync / scalar / gpsimd).
- Per layer: LN1 -> QKV -> AllGather K,V (bf16) across the batch group
  (next layer's weights prefetched at layer start) -> causal attention,
  per-head PSUM accumulation, masks multiplied on the vector engine,
  per-head pipelined softmax normalization -> Wo + residual -> LN2 ->
  MLP -> residual.
- Final LN -> AllGather hidden (8-core, Shared/RDH) overlapped with
  discarded warmup matmuls on the local tokens -> vocab-sharded LM head
  with Wlm resident in SBUF (loaded in 4 slices during layers 0-2);
  logits accumulate 13 vocab tiles in SBUF and go out as one 13KB-per-
  partition DMA per 128-token group, alternating queues.
- Host folds LN gamma/beta and the 1/sqrt(HS) score scale into the
  weights; embedding gather happens host-side (tiny).
"""

import numpy as np
import ml_dtypes

import concourse.bass as bass
import concourse.tile as tile
from concourse import bacc, mybir
from concourse import bass_utils
from concourse.bass import ds, ts
from concourse.masks import make_identity

FP = mybir.dt.float32
BF = mybir.dt.bfloat16
AF = mybir.ActivationFunctionType
OP = mybir.AluOpType

V, D, T, L, H, HS, B = 50257, 512, 1024, 6, 8, 64, 2
FF = 4 * D
EPS = 1e-5
NC = 8          # cores
CH = 256        # tokens per core
VS = 6284       # padded vocab shard per core; 8*VS = 50272 >= V
KD = D // 128   # 4 k-tiles over D
MD = FF // 128  # 16 m-tiles over FF
BT = B * T
NT = (VS + 511) // 512   # 13 vocab tiles per core
NTP = NT * 512           # padded vocab shard (6656)


def build_program(with_bias=True, layers=L):
    nc = bacc.Bacc("TRN2", target_bir_lowering=False, debug=False, num_devices=NC)

    # ---- I/O (weights host-side pre-rearranged to partition-major) ----
    x0 = nc.dram_tensor("x0", [128, 2, D], FP, kind="ExternalInput").ap()
    wq = nc.dram_tensor("wq", [L, 128, KD, D], BF, kind="ExternalInput").ap()
    wk = nc.dram_tensor("wk", [L, 128, KD, D], BF, kind="ExternalInput").ap()
    wv = nc.dram_tensor("wv", [L, 128, KD, D], BF, kind="ExternalInput").ap()
    wo = nc.dram_tensor("wo", [L, 128, KD, D], BF, kind="ExternalInput").ap()
    w1 = nc.dram_tensor("w1", [L, 128, KD, FF], BF, kind="ExternalInput").ap()
    w2 = nc.dram_tensor("w2", [L, 128, MD, D], BF, kind="ExternalInput").ap()
    wlm = nc.dram_tensor("wlm", [128, KD, VS], BF, kind="ExternalInput").ap()
    bqk = nc.dram_tensor("bqk", [128, L, 2, KD], FP, kind="ExternalInput").ap()
    b1t = nc.dram_tensor("b1t", [128, L, MD], FP, kind="ExternalInput").ap()
    bo2 = nc.dram_tensor("bo2", [L, 2, D], FP, kind="ExternalInput").ap()
    # causal 0/1 mask per core: [p, kchunk, ktile, 256 queries] bf16
    msk = nc.dram_tensor("msk", [128, 4, 2, CH], BF, kind="ExternalInput").ap()
    # logits row (tg, p) = token tg*128+p of the gathered order
    logits = nc.dram_tensor("logits", [BT // 128, 128, NTP], BF,
                            kind="ExternalOutput").ap()

    KV_K = 128 * KD * CH            # kT flat elements per core
    KV_V = 128 * 2 * H * 65         # v_aug flat elements per core
    KV = KV_K + KV_V
    XF = 128 * KD * CH              # xfT flat elements

    from contextlib import ExitStack
    with ExitStack() as stk:
        tc = stk.enter_context(tile.TileContext(nc))
        ec = stk.enter_context
        consts = ec(tc.tile_pool(name="consts", bufs=1))
        wlmp = ec(tc.tile_pool(name="wlmp", bufs=1))
        xpool = ec(tc.tile_pool(name="xpool", bufs=1))
        hpool = ec(tc.tile_pool(name="hpool", bufs=2))
        t4 = ec(tc.tile_pool(name="t4", bufs=4))          # [128,KD,CH] transposed acts
        wqkvop = ec(tc.tile_pool(name="wqkvo", bufs=2))   # [128,KD,4,512]
        w1pool = ec(tc.tile_pool(name="w1k", bufs=3))     # [128,KD,FF//2] halves
        w2pool = ec(tc.tile_pool(name="w2k", bufs=3))     # [128,MD//2,D] halves
        kvall = ec(tc.tile_pool(name="kvall", bufs=1))
        vaugp = ec(tc.tile_pool(name="vaug", bufs=1))
        small = ec(tc.tile_pool(name="small", bufs=2))
        expp = ec(tc.tile_pool(name="exp", bufs=2))
        gtp = ec(tc.tile_pool(name="gt", bufs=2))
        attp = ec(tc.tile_pool(name="attp", bufs=1))
        rbcp = ec(tc.tile_pool(name="rbc", bufs=2))
        lgp = ec(tc.tile_pool(name="lg", bufs=2))
        bcp = ec(tc.tile_pool(name="bcast", bufs=2))
        xftp = ec(tc.tile_pool(name="xft", bufs=1))
        xfap = ec(tc.tile_pool(name="xfa", bufs=2))
        mmp = ec(tc.tile_pool(name="mm", bufs=2, space="PSUM"))
        avp = ec(tc.tile_pool(name="avp", bufs=2, space="PSUM"))
        spp = ec(tc.tile_pool(name="sp", bufs=2, space="PSUM"))
        mop = ec(tc.tile_pool(name="mo", bufs=2, space="PSUM"))
        dram = ec(tc.tile_pool(name="dram", bufs=2, space="DRAM"))

        # ---- consts + first activations on the sync queue ----
        xt = xpool.tile([128, 2, D], FP, tag="xt")
        nc.sync.dma_start(xt[:], x0)
        ident = consts.tile([128, 128], BF)
        make_identity(nc, ident)
        epst = consts.tile([128, 1], FP)
        nc.vector.memset(epst, EPS)
        ones_sb = consts.tile([128, 64], BF)
        nc.vector.memset(ones_sb, 1.0)
        msk_sb = consts.tile([128, 4, 2, CH], BF)
        nc.scalar.dma_start(msk_sb[:], msk)
        bqk_sb = consts.tile([128, L, 2, KD], FP)
        b1_sb = consts.tile([128, L, MD], FP)
        if with_bias:
            nc.scalar.dma_start(bqk_sb[:], bqk)
            nc.scalar.dma_start(b1_sb[:], b1t)

        # ---- layer-0 weights (scalar + gpsimd queues), then resident Wlm --
        def load_wqkvo(l):
            w = wqkvop.tile([128, KD, 4, 512], BF, tag="wqkvo", name=f"wqkvo{l}")
            # k first (needed first each layer), v first on the other queue
            for i, srct, eng in ((1, wk, nc.scalar), (0, wq, nc.scalar),
                                 (2, wv, nc.gpsimd), (3, wo, nc.gpsimd)):
                eng.dma_start(w[:, :, i, :], srct[l])
            return w

        def load_w1h(l, i):
            w = w1pool.tile([128, KD, FF // 2], BF, tag="w1k",
                            name=f"w1k{l}_{i}")
            nc.scalar.dma_start(w[:], w1[l, :, :, ds(i * (FF // 2), FF // 2)])
            return w

        def load_w2h(l, i):
            w = w2pool.tile([128, MD // 2, D], BF, tag="w2k",
                            name=f"w2k{l}_{i}")
            nc.gpsimd.dma_start(w[:], w2[l, :, ds(i * (MD // 2), MD // 2), :])
            return w

        cur_wqkvo = load_wqkvo(0)
        cur_w1 = [load_w1h(0, 0), load_w1h(0, 1)]
        cur_w2 = [load_w2h(0, 0), load_w2h(0, 1)]
        wlm_sb = wlmp.tile([128, KD, VS], BF)
        nc.gpsimd.dma_start(wlm_sb[:, 0, :], wlm[:, 0, :])

        def layernorm_tt(src, out, tt):
            """normalize src [128,2,D] token-half tt into out (bf16)."""
            st = small.tile([128, 6], FP, tag="bnst")
            nc.vector.bn_stats(st[:], src[:, tt, :])
            mv = small.tile([128, 2], FP, tag="bnmv")
            nc.vector.bn_aggr(mv[:], st[:])
            nc.scalar.activation(mv[:, 1:2], mv[:, 1:2], AF.Sqrt,
                                 bias=epst[:, 0:1])
            nc.vector.reciprocal(mv[:, 1:2], mv[:, 1:2])
            nc.vector.tensor_scalar(
                out=out[:, tt, :], in0=src[:, tt, :],
                scalar1=mv[:, 0:1], scalar2=mv[:, 1:2],
                op0=OP.subtract, op1=OP.mult)

        def layernorm(src):
            out = hpool.tile([128, 2, D], BF, tag="h")
            for tt in range(2):
                layernorm_tt(src, out, tt)
            return out

        def transpose2(src, pool=t4):
            """src [128,2,D] bf16 (tokens, dims) -> [128,KD,CH] (dims, toks)."""
            out = pool.tile([128, KD, CH], BF, tag="t4" if pool is t4 else "xft")
            # tt outer: all four tt=0 transposes issue before any instruction
            # that waits on LN(tt=1) - the in-order PE never blocks early
            for tt in range(2):
                for d in range(KD):
                    ps = mmp.tile([128, 1024], BF, tag="mm")
                    nc.tensor.transpose(ps[:, 0:128], src[:, tt, ds(d * 128, 128)],
                                        ident[:])
                    nc.vector.tensor_copy(out[:, d, ds(tt * 128, 128)],
                                          ps[:, 0:128])
            return out

        nxt_wqkvo = nxt_w1h = nxt_w2h = None
        h_cur = layernorm(xt)
        for l in range(layers):
            # ---- prefetch next layer's weights + one Wlm slice ----
            if l + 1 < layers:
                nxt_wqkvo = load_wqkvo(l + 1)
                nxt_w1h = load_w1h(l + 1, 0)
                nxt_w2h = load_w2h(l + 1, 0)
            if l < KD - 1:
                nc.gpsimd.dma_start(wlm_sb[:, l + 1, :], wlm[:, l + 1, :])

            # ---- LN1 (already computed) + transpose ----
            hT = transpose2(h_cur)

            # ---- gather hT itself (smallest possible payload); K/V for
            # all four chunks are recomputed locally after the gather, which
            # also re-warms the PE clock before attention ----
            ht_in = dram.tile([XF], BF, tag="htin")
            nc.sync.dma_start(
                ht_in[:].rearrange("(p a b) -> p a b", p=128, a=KD), hT[:])
            ht_out = dram.tile([4, XF], BF, tag="htout")
            nc.gpsimd.collective_compute(
                "AllGather", OP.bypass,
                replica_groups=[[0, 1, 2, 3], [4, 5, 6, 7]],
                ins=[ht_in[:].opt()], outs=[ht_out[:].opt()])

            # ---- q projection overlaps the collective ----
            qT = t4.tile([128, KD, CH], BF, tag="t4")
            for d in range(KD):
                ps = mmp.tile([128, 512], FP, tag="mm")
                for k in range(KD):
                    nc.tensor.matmul(ps[:, :CH], cur_wqkvo[:, k, 0, ds(d * 128, 128)],
                                     hT[:, k, :], start=(k == 0),
                                     stop=(k == KD - 1))
                if with_bias:
                    nc.vector.tensor_scalar_add(qT[:, d, :], ps[:, :CH],
                                                bqk_sb[:, l, 0, d:d + 1])
                else:
                    nc.vector.tensor_copy(qT[:, d, :], ps[:, :CH])
            # dense discarded matmul trains: whenever the PE would idle out
            # the HAM busy-window (collective waits, ACT-bound stretches) we
            # feed it junk matmuls so subsequent real ones run at full clock
            def dummy_train(n):
                for dd in range(n):
                    ps = mmp.tile([128, 512], FP, tag="mm")
                    for k in range(KD):
                        nc.tensor.matmul(ps[:, :CH], cur_wqkvo[:, k, 0, 0:128],
                                         hT[:, k, :], start=(k == 0),
                                         stop=(k == KD - 1))
            dummy_train(22)

            hTall = kvall.tile([128, KD, 4, CH], BF, tag="htall")
            for c in range(4):
                (nc.sync if c % 2 == 0 else nc.scalar).dma_start(
                    hTall[:, :, c, :],
                    ht_out[c].rearrange("(p a b) -> p a b", p=128, a=KD))

            # ---- K (transposed) and V for all four chunks ----
            kTall = kvall.tile([128, KD, 4, CH], BF, tag="ktall")
            vall = kvall.tile([128, 4, 2, H, 65], BF, tag="vall")
            nc.vector.memset(vall[:, :, :, :, 64:65], 1.0)
            for cp in range(2):
                for d in range(KD):
                    ps = mmp.tile([128, 512], FP, tag="mm")
                    for k in range(KD):
                        nc.tensor.matmul(
                            ps[:], cur_wqkvo[:, k, 1, ds(d * 128, 128)],
                            hTall[:, k, ds(cp * 2, 2), :], start=(k == 0),
                            stop=(k == KD - 1))
                    if with_bias:
                        nc.vector.tensor_scalar_add(
                            kTall[:, d, ds(cp * 2, 2), :],
                            ps[:].rearrange("p (c b) -> p c b", c=2),
                            bqk_sb[:, l, 1, d:d + 1])
                    else:
                        nc.vector.tensor_copy(
                            kTall[:, d, ds(cp * 2, 2), :],
                            ps[:].rearrange("p (c b) -> p c b", c=2))
                for ci in range(2):
                    c = cp * 2 + ci
                    for tt in range(2):
                        ps = mmp.tile([128, 512], FP, tag="mm")
                        for k in range(KD):
                            nc.tensor.matmul(
                                ps[:, :D], hTall[:, k, c, ds(tt * 128, 128)],
                                cur_wqkvo[:, k, 2, :], start=(k == 0),
                                stop=(k == KD - 1))
                        nc.vector.tensor_copy(
                            vall[:, c, tt, :, 0:64],
                            ps[:].rearrange("p (h e) -> p h e", h=H))

            # ---- attention: per-head PSUM accumulation over the 4 chunks.
            # attT is assembled with RAW (unnormalized) values; denominators
            # collect in dram, then one compact reciprocal + a broadcast DMA
            # into transposed layout + a single in-place multiply normalize.
            attT = t4.tile([128, KD, CH], BF, tag="t4")
            # araw rows 0-63 = numerators; row 64 = denominators
            araw = attp.tile([128, H, CH], BF, tag="araw", name=f"araw{l}")
            rbcT = rbcp.tile([128, KD, CH], BF, tag="rbcT", name=f"rbcT{l}")
            wops = [mop.tile([128, D], FP, tag="mo", name=f"wo{l}_{t}")
                    for t in range(2)]
            normed = set()

            def norm_dt(dt2):
                """normalize a dt column, then immediately fold its Wo
                contribution in - real PE work amid the attention stream.
                The denominator broadcast is two tiny PE matmuls (ones-row
                outer product) instead of a DMA round trip."""
                if dt2 in normed or dt2 >= KD:
                    return
                normed.add(dt2)
                bc = mmp.tile([128, 512], FP, tag="mm")
                for pa in range(2):
                    nc.tensor.matmul(bc[pa * 64:(pa + 1) * 64, 0:CH],
                                     ones_sb[64:65, :],
                                     araw[64:65, 2 * dt2 + pa, :],
                                     start=True, stop=True)
                with nc.allow_low_precision(reason="softmax denom recip bf16"):
                    nc.vector.reciprocal(rbcT[:, dt2, :], bc[:, 0:CH])
                nc.vector.tensor_tensor(attT[:, dt2, :], attT[:, dt2, :],
                                        rbcT[:, dt2, :], OP.mult)
                for tt in range(2):
                    nc.tensor.matmul(wops[tt][:],
                                     attT[:, dt2, ds(tt * 128, 128)],
                                     cur_wqkvo[:, dt2, 3, :],
                                     start=(dt2 == 0), stop=(dt2 == KD - 1))

            for hp in range(H // 2):
                avt = {}
                for hh in (2 * hp, 2 * hp + 1):
                    avt[hh] = avp.tile([65, CH], FP, tag="av", name=f"av{l}_{hh}")
                for c in range(4):
                    ext = {}
                    for hh in (2 * hp, 2 * hp + 1):
                        pb = (hh % 2) * 64
                        sps = spp.tile([128, 2, CH], FP, tag="sp")
                        for kt in range(2):
                            nc.tensor.matmul(
                                sps[:, kt, :],
                                kTall[pb:pb + 64, hp, c, ds(kt * 128, 128)],
                                qT[pb:pb + 64, hp, :],
                                start=True, stop=True)
                        ex = expp.tile([128, 2, CH], BF, tag="exp")
                        nc.scalar.activation(ex[:], sps[:], AF.Exp)
                        # one of four mask-mults rides on gpsimd
                        eng = nc.gpsimd if (c == 3 and hh % 2 == 0) else nc.vector
                        eng.tensor_tensor(ex[:], ex[:], msk_sb[:, c, :, :],
                                          OP.mult)
                        ext[hh] = ex
                    for hh in (2 * hp, 2 * hp + 1):
                        for kt in range(2):
                            nc.tensor.matmul(
                                avt[hh][:], vall[:, c, kt, hh, :],
                                ext[hh][:, kt, :],
                                start=(c == 0 and kt == 0),
                                stop=(c == 3 and kt == 1))
                for hh in (2 * hp, 2 * hp + 1):
                    pb = (hh % 2) * 64
                    avps = avt[hh]
                    # eagerly drain PSUM so later heads are never blocked
                    if pb == 0:
                        nc.vector.tensor_copy(attT[0:64, hp, :], avps[0:64, :])
                        nc.vector.tensor_copy(araw[64:65, hh, :], avps[64:65, :])
                    else:
                        nc.vector.tensor_copy(araw[0:65, hh, :], avps[0:65, :])
                        nc.scalar.dma_start(attT[64:128, hp, :], araw[0:64, hh, :])
                # normalize a dt-column one pair after its denominators landed
                if hp >= 1:
                    norm_dt(hp - 1)
            norm_dt(3)
            dummy_train(3)

            # ---- Wo accumulated during attention: bias + residual + LN2 ----
            h2 = hpool.tile([128, 2, D], BF, tag="h")
            if with_bias:
                bo_b = bcp.tile([128, D], FP, tag="bc")
                bo_src = bo2[l, 0]
                nc.sync.dma_start(bo_b[:], bass.AP(
                    tensor=bo_src.tensor, offset=bo_src.offset,
                    ap=[[0, 128]] + list(bo_src.ap)))
            for tt in range(2):
                if with_bias:
                    nc.vector.tensor_tensor(wops[tt][:], wops[tt][:], bo_b[:],
                                            OP.add)
                nc.vector.tensor_tensor(xt[:, tt, :], xt[:, tt, :],
                                        wops[tt][:], OP.add)
                layernorm_tt(xt, h2, tt)

            # ---- LN2 + transpose ----
            h2T = transpose2(h2)

            # ---- MLP ----
            if with_bias:
                b2_b = bcp.tile([128, D], FP, tag="bc")
                b2_src = bo2[l, 1]
                nc.sync.dma_start(b2_b[:], bass.AP(
                    tensor=b2_src.tensor, offset=b2_src.offset,
                    ap=[[0, 128]] + list(b2_src.ap)))
            x2ps = [mop.tile([128, D], FP, tag="mo", name=f"mo{l}_{kk}")
                    for kk in range(2)]
            for mp in range(MD // 2):
                gps = mmp.tile([128, 2, CH], FP, tag="mm")
                for mi in range(2):
                    m = mp * 2 + mi
                    w1h = cur_w1[m // (MD // 2)]
                    mm = m % (MD // 2)
                    for k in range(KD):
                        nc.tensor.matmul(gps[:, mi, :],
                                         w1h[:, k, ds(mm * 128, 128)],
                                         h2T[:, k, :], start=(k == 0),
                                         stop=(k == KD - 1))
                gt = gtp.tile([128, 2, CH], BF, tag="gt")
                if with_bias:
                    for mi in range(2):
                        m = mp * 2 + mi
                        nc.scalar.activation(gt[:, mi, :], gps[:, mi, :], AF.Gelu,
                                             bias=b1_sb[:, l, m:m + 1])
                else:
                    nc.scalar.activation(gt[:], gps[:], AF.Gelu)
                for mi in range(2):
                    m = mp * 2 + mi
                    w2h = cur_w2[m // (MD // 2)]
                    for tt in range(2):
                        nc.tensor.matmul(x2ps[tt][:], gt[:, mi, ds(tt * 128, 128)],
                                         w2h[:, m % (MD // 2), :],
                                         start=(m == 0), stop=(m == MD - 1))
                if mp == 3 and l + 1 < layers:
                    # the current layer's first halves had their last read at
                    # m=7 (this iteration) - their ring slots are now free
                    nxt_w1 = [nxt_w1h, load_w1h(l + 1, 1)]
                    nxt_w2 = [nxt_w2h, load_w2h(l + 1, 1)]
            h_nxt = hpool.tile([128, 2, D], BF, tag="h")
            for tt in range(2):
                if with_bias:
                    nc.vector.tensor_tensor(x2ps[tt][:], x2ps[tt][:], b2_b[:],
                                            OP.add)
                nc.vector.tensor_tensor(xt[:, tt, :], xt[:, tt, :], x2ps[tt][:],
                                        OP.add)
                layernorm_tt(xt, h_nxt, tt)
            h_cur = h_nxt
            if l + 1 < layers:
                cur_wqkvo, cur_w1, cur_w2 = nxt_wqkvo, nxt_w1, nxt_w2

        # ---- final LN; AllGather hidden state across all 8 (Shared/RDH) ----
        xfT = transpose2(h_cur, pool=xftp)
        xf_in = dram.tile([XF], BF, tag="xfin")
        nc.sync.dma_start(
            xf_in[:].rearrange("(p a b) -> p a b", p=128, a=KD), xfT[:])
        xf_out = dram.tile([NC, XF], BF, tag="xfout", addr_space="Shared")
        nc.gpsimd.collective_compute(
            "AllGather", OP.bypass,
            replica_groups=[list(range(NC))],
            ins=[xf_in[:].opt()], outs=[xf_out[:].opt()])

        hp_pools = ((mmp, "mm"), (mop, "mo"), (spp, "sp"))
        dma_engs = (nc.sync, nc.scalar, nc.gpsimd)
        gi = 0
        LGB = [0, 5, 9, 13]  # logits go out in chunks of 5/4/4 vocab tiles

        def head_group(get_lhs, tg):
            """One 128-token group: all NT vocab tiles -> three chunked DMAs.
            tg None => warmup-only (compute and discard)."""
            nonlocal gi
            lgt = None
            cj = 0
            for n in range(NT):
                if tg is not None and n == LGB[cj]:
                    lgt = lgp.tile([128, 5, 512], BF, tag="lg",
                                   name=f"lg{tg}_{cj}")
                nsz = min(512, VS - n * 512)
                pool_i, ptag = hp_pools[gi % 3]
                ps = pool_i.tile([128, 512], FP, tag=ptag,
                                 name=f"hps{tg}_{n}")
                for k in range(KD):
                    nc.tensor.matmul(
                        ps[:, :nsz], get_lhs(k),
                        wlm_sb[:, k, ds(n * 512, nsz)],
                        start=(k == 0), stop=(k == KD - 1))
                gi += 1
                if tg is None:
                    continue
                nb = n - LGB[cj]
                if gi % 2 == 0:
                    nc.vector.tensor_copy(lgt[:, nb, :nsz], ps[:, :nsz])
                else:
                    nc.scalar.activation(lgt[:, nb, :nsz], ps[:, :nsz], AF.Copy)
                if n == LGB[cj + 1] - 1:
                    nw = LGB[cj + 1] - LGB[cj]
                    dma_engs[(tg + cj) % 3].dma_start(
                        logits[tg, :, ds(LGB[cj] * 512, nw * 512)],
                        lgt[:, 0:nw, :])
                    cj += 1

        # pass 1: warmup on local tokens while the AllGather flies (discard)
        for mt in range(2):
            head_group(lambda k, mt=mt: xfT[:, k, ds(mt * 128, 128)], None)

        # read the gathered slots back pairwise (ring of 2 tiles) so the
        # next pair's DMAs overlap the current pair's matmuls
        for p in range(4):
            xa = xfap.tile([128, KD, 2, CH], BF, tag="xfa", name=f"xfa{p}")
            for ci in range(2):
                dma_engs[(p * 2 + ci) % 3].dma_start(
                    xa[:, :, ci, :],
                    xf_out[p * 2 + ci, :].rearrange("(p a b) -> p a b",
                                                    p=128, a=KD))
            for ci in range(2):
                c = p * 2 + ci
                for mt in range(2):
                    head_group(
                        lambda k, xa=xa, ci=ci, mt=mt:
                            xa[:, k, ci, ds(mt * 128, 128)],
                        c * 2 + mt)

    nc.compile()
    return nc


_CACHE = {}


def _get_program(with_bias=True):
    key = ("nc", with_bias)
    if key not in _CACHE:
        _CACHE[key] = build_program(with_bias=with_bias)
    return _CACHE[key]


def _to_pmaj(w):
    """[.., D_in, N] -> [.., 128, D_in//128, N] partition-major."""
    shp = w.shape
    kd = shp[-2] // 128
    return np.ascontiguousarray(
        w.reshape(*shp[:-2], kd, 128, shp[-1]).swapaxes(-3, -2))


def _prep_inputs(inputs):
    f = lambda k: np.asarray(inputs[k], np.float32)
    bf = ml_dtypes.bfloat16
    idx = np.asarray(inputs["idx"]).astype(np.int64)
    tok_emb, pos_emb = f("tok_emb"), f("pos_emb")
    x0 = tok_emb[idx] + pos_emb[None, :T]          # [B, T, D]
    x0 = x0.reshape(NC, 2, 128, D).transpose(0, 2, 1, 3).copy()  # [NC,128,2,D]

    ln1_g, ln1_b = f("ln1_g"), f("ln1_b")
    ln2_g, ln2_b = f("ln2_g"), f("ln2_b")
    Wq, bq = f("Wq"), f("bq")
    Wk, bk = f("Wk"), f("bk")
    Wv, bv = f("Wv"), f("bv")
    Wo, bo = f("Wo"), f("bo")
    W1, b1 = f("W1"), f("b1")
    W2, b2 = f("W2"), f("b2")
    lnf_g, lnf_b = f("lnf_g"), f("lnf_b")
    Wlm, blm = f("Wlm"), f("blm")

    sc = 1.0 / np.sqrt(HS)
    wqe = ln1_g[:, :, None] * Wq * sc
    bqe = (np.einsum("ld,ldm->lm", ln1_b, Wq) + bq) * sc
    wke = ln1_g[:, :, None] * Wk
    bke = np.einsum("ld,ldm->lm", ln1_b, Wk) + bk
    wve = ln1_g[:, :, None] * Wv
    bve = np.einsum("ld,ldm->lm", ln1_b, Wv) + bv
    boe = np.einsum("lm,lmd->ld", bve, Wo) + bo
    w1e = ln2_g[:, :, None] * W1
    b1e = np.einsum("ld,ldf->lf", ln2_b, W1) + b1
    wlme = lnf_g[:, None] * Wlm
    blme = lnf_b @ Wlm + blm

    bqk = np.stack([bqe, bke], axis=1)             # [L, 2, D]
    bqk = bqk.reshape(L, 2, KD, 128).transpose(3, 0, 1, 2).copy()
    b1t = b1e.reshape(L, MD, 128).transpose(2, 0, 1).copy()
    bo2 = np.stack([boe, b2], axis=1)              # [L, 2, D]

    wlmp = np.zeros((D, NC * VS), np.float32)
    wlmp[:, :V] = wlme

    masks = []
    for core in range(NC):
        cc = core % 4
        qpos = cc * CH + np.arange(CH)
        m = np.empty((128, 4, 2, CH), np.float32)
        for kc in range(4):
            for kt in range(2):
                kpos = kc * CH + kt * 128 + np.arange(128)
                m[:, kc, kt, :] = (kpos[:, None] <= qpos[None, :]).astype(np.float32)
        masks.append(m.astype(bf))

    shared = dict(wq=_to_pmaj(wqe.astype(bf)), wk=_to_pmaj(wke.astype(bf)),
                  wv=_to_pmaj(wve.astype(bf)), wo=_to_pmaj(Wo.astype(bf)),
                  w1=_to_pmaj(w1e.astype(bf)), w2=_to_pmaj(W2.astype(bf)),
                  bqk=bqk, b1t=b1t, bo2=np.ascontiguousarray(bo2))
    in_maps = []
    for core in range(NC):
        m = dict(shared)
        m["x0"] = np.ascontiguousarray(x0[core])
        m["msk"] = masks[core]
        m["wlm"] = _to_pmaj(wlmp[:, core * VS:(core + 1) * VS].astype(bf))
        in_maps.append(m)
    return in_maps, blme


def _run(inputs, trace=False):
    in_maps, blme = _prep_inputs(inputs)
    with_bias = bool(np.any(in_maps[0]["bo2"]))
    nc = _get_program(with_bias=with_bias)
    res = bass_utils.run_bass_kernel_spmd(nc, in_maps, core_ids=list(range(NC)),
                                          trace=trace)
    lg = np.concatenate(
        [np.asarray(res.results[c]["logits"]).astype(np.float32)
         .reshape(BT, NTP)[:, :VS] for c in range(NC)], axis=1)
    out = lg[:, :V]
    if np.any(blme):
        out = out + blme[None, :]
    return out.reshape(B, T, V).astype(np.float32), res


def kernel(**inputs) -> np.ndarray:
    out, _ = _run(inputs, trace=False)
    return out


# revision 46
# speedup vs baseline: 1.0332x; 1.0332x over previous
"""Trainium2 Bass kernel for a 6-layer GPT (D=512, H=8, T=1024, B=2, V=50257).

Strategy (8 NeuronCores), v3:
- Token-shard the transformer body: core c owns 256 tokens (cores 0-3 =
  batch 0 chunks 0-3, cores 4-7 = batch 1 chunks 0-3).
- All matmul operands bf16 (PSUM accumulates fp32); residual/LN fp32.
- Host pre-rearranges every weight into partition-major layout so DMA
  descriptors are >=4KB contiguous lines; DMAs are spread over the three
  trigger queues (sync / scalar / gpsimd).
- Per layer: LN1 -> QKV -> AllGather K,V (bf16) across the batch group
  (next layer's weights prefetched at layer start) -> causal attention,
  per-head PSUM accumulation, masks multiplied on the vector engine,
  per-head pipelined softmax normalization -> Wo + residual -> LN2 ->
  MLP -> residual.
- Final LN -> AllGather hidden (8-core, Shared/RDH) overlapped with
  discarded warmup matmuls on the local tokens -> vocab-sharded LM head
  with Wlm resident in SBUF (loaded in 4 slices during layers 0-2);
  logits accumulate 13 vocab tiles in SBUF and go out as one 13KB-per-
  partition DMA per 128-token group, alternating queues.
- Host folds LN gamma/beta and the 1/sqrt(HS) score scale into the
  weights; embedding gather happens host-side (tiny).
"""

import numpy as np
import ml_dtypes

import concourse.bass as bass
import concourse.tile as tile
from concourse import bacc, mybir
from concourse import bass_utils
from concourse.bass import ds, ts
from concourse.masks import make_identity

FP = mybir.dt.float32
BF = mybir.dt.bfloat16
AF = mybir.ActivationFunctionType
OP = mybir.AluOpType

V, D, T, L, H, HS, B = 50257, 512, 1024, 6, 8, 64, 2
FF = 4 * D
EPS = 1e-5
NC = 8          # cores
CH = 256        # tokens per core
VS = 6284       # padded vocab shard per core; 8*VS = 50272 >= V
KD = D // 128   # 4 k-tiles over D
MD = FF // 128  # 16 m-tiles over FF
BT = B * T
NT = (VS + 511) // 512   # 13 vocab tiles per core
NTP = NT * 512           # padded vocab shard (6656)


def build_program(with_bias=True, layers=L):
    nc = bacc.Bacc("TRN2", target_bir_lowering=False, debug=False, num_devices=NC)

    # ---- I/O (weights host-side pre-rearranged to partition-major) ----
    x0 = nc.dram_tensor("x0", [128, 2, D], FP, kind="ExternalInput").ap()
    wq = nc.dram_tensor("wq", [L, 128, KD, D], BF, kind="ExternalInput").ap()
    wk = nc.dram_tensor("wk", [L, 128, KD, D], BF, kind="ExternalInput").ap()
    wv = nc.dram_tensor("wv", [L, 128, KD, D], BF, kind="ExternalInput").ap()
    wo = nc.dram_tensor("wo", [L, 128, KD, D], BF, kind="ExternalInput").ap()
    w1 = nc.dram_tensor("w1", [L, 128, KD, FF], BF, kind="ExternalInput").ap()
    w2 = nc.dram_tensor("w2", [L, 128, MD, D], BF, kind="ExternalInput").ap()
    wlm = nc.dram_tensor("wlm", [128, KD, VS], BF, kind="ExternalInput").ap()
    bqk = nc.dram_tensor("bqk", [128, L, 2, KD], FP, kind="ExternalInput").ap()
    b1t = nc.dram_tensor("b1t", [128, L, MD], FP, kind="ExternalInput").ap()
    bo2 = nc.dram_tensor("bo2", [L, 2, D], FP, kind="ExternalInput").ap()
    # causal 0/1 mask per core: [p, kchunk, ktile, 256 queries] bf16
    msk = nc.dram_tensor("msk", [128, 4, 2, CH], BF, kind="ExternalInput").ap()
    # logits row (tg, p) = token tg*128+p of the gathered order
    logits = nc.dram_tensor("logits", [BT // 128, 128, NTP], BF,
                            kind="ExternalOutput").ap()

    KV_K = 128 * KD * CH            # kT flat elements per core
    KV_V = 128 * 2 * H * 65         # v_aug flat elements per core
    KV = KV_K + KV_V
    XF = 128 * KD * CH              # xfT flat elements

    from contextlib import ExitStack
    with ExitStack() as stk:
        tc = stk.enter_context(tile.TileContext(nc))
        ec = stk.enter_context
        consts = ec(tc.tile_pool(name="consts", bufs=1))
        wlmp = ec(tc.tile_pool(name="wlmp", bufs=1))
        xpool = ec(tc.tile_pool(name="xpool", bufs=1))
        hpool = ec(tc.tile_pool(name="hpool", bufs=2))
        t4 = ec(tc.tile_pool(name="t4", bufs=4))          # [128,KD,CH] transposed acts
        wqkvop = ec(tc.tile_pool(name="wqkvo", bufs=2))   # [128,KD,4,512]
        w1pool = ec(tc.tile_pool(name="w1k", bufs=3))     # [128,KD,FF//2] halves
        w2pool = ec(tc.tile_pool(name="w2k", bufs=3))     # [128,MD//2,D] halves
        kvall = ec(tc.tile_pool(name="kvall", bufs=1))
        vaugp = ec(tc.tile_pool(name="vaug", bufs=1))
        small = ec(tc.tile_pool(name="small", bufs=2))
        expp = ec(tc.tile_pool(name="exp", bufs=3))
        gtp = ec(tc.tile_pool(name="gt", bufs=2))
        attp = ec(tc.tile_pool(name="attp", bufs=1))
        rbcp = ec(tc.tile_pool(name="rbc", bufs=2))
        lgp = ec(tc.tile_pool(name="lg", bufs=2))
        bcp = ec(tc.tile_pool(name="bcast", bufs=2))
        xftp = ec(tc.tile_pool(name="xft", bufs=1))
        xfap = ec(tc.tile_pool(name="xfa", bufs=2))
        mmp = ec(tc.tile_pool(name="mm", bufs=2, space="PSUM"))
        avp = ec(tc.tile_pool(name="avp", bufs=2, space="PSUM"))
        spp = ec(tc.tile_pool(name="sp", bufs=2, space="PSUM"))
        mop = ec(tc.tile_pool(name="mo", bufs=2, space="PSUM"))
        dram = ec(tc.tile_pool(name="dram", bufs=2, space="DRAM"))

        # ---- consts + first activations on the sync queue ----
        xt = xpool.tile([128, 2, D], FP, tag="xt")
        nc.sync.dma_start(xt[:], x0)
        ident = consts.tile([128, 128], BF)
        make_identity(nc, ident)
        epst = consts.tile([128, 1], FP)
        nc.vector.memset(epst, EPS)
        ones_sb = consts.tile([128, 64], BF)
        nc.vector.memset(ones_sb, 1.0)
        msk_sb = consts.tile([128, 4, 2, CH], BF)
        nc.scalar.dma_start(msk_sb[:], msk)
        bqk_sb = consts.tile([128, L, 2, KD], FP)
        b1_sb = consts.tile([128, L, MD], FP)
        if with_bias:
            nc.scalar.dma_start(bqk_sb[:], bqk)
            nc.scalar.dma_start(b1_sb[:], b1t)

        # ---- layer-0 weights (scalar + gpsimd queues), then resident Wlm --
        def load_wqkvo(l):
            w = wqkvop.tile([128, KD, 4, 512], BF, tag="wqkvo", name=f"wqkvo{l}")
            # k first (needed first each layer), v first on the other queue
            for i, srct, eng in ((1, wk, nc.scalar), (0, wq, nc.scalar),
                                 (2, wv, nc.gpsimd), (3, wo, nc.gpsimd)):
                eng.dma_start(w[:, :, i, :], srct[l])
            return w

        def load_w1h(l, i):
            w = w1pool.tile([128, KD, FF // 2], BF, tag="w1k",
                            name=f"w1k{l}_{i}")
            nc.scalar.dma_start(w[:], w1[l, :, :, ds(i * (FF // 2), FF // 2)])
            return w

        def load_w2h(l, i):
            w = w2pool.tile([128, MD // 2, D], BF, tag="w2k",
                            name=f"w2k{l}_{i}")
            nc.gpsimd.dma_start(w[:], w2[l, :, ds(i * (MD // 2), MD // 2), :])
            return w

        cur_wqkvo = load_wqkvo(0)
        cur_w1 = [load_w1h(0, 0), load_w1h(0, 1)]
        cur_w2 = [load_w2h(0, 0), load_w2h(0, 1)]
        wlm_sb = wlmp.tile([128, KD, VS], BF)
        nc.gpsimd.dma_start(wlm_sb[:, 0, :], wlm[:, 0, :])

        def layernorm_tt(src, out, tt):
            """normalize src [128,2,D] token-half tt into out (bf16)."""
            st = small.tile([128, 6], FP, tag="bnst")
            nc.vector.bn_stats(st[:], src[:, tt, :])
            mv = small.tile([128, 2], FP, tag="bnmv")
            nc.vector.bn_aggr(mv[:], st[:])
            nc.scalar.activation(mv[:, 1:2], mv[:, 1:2], AF.Sqrt,
                                 bias=epst[:, 0:1])
            nc.vector.reciprocal(mv[:, 1:2], mv[:, 1:2])
            nc.vector.tensor_scalar(
                out=out[:, tt, :], in0=src[:, tt, :],
                scalar1=mv[:, 0:1], scalar2=mv[:, 1:2],
                op0=OP.subtract, op1=OP.mult)

        def layernorm(src):
            out = hpool.tile([128, 2, D], BF, tag="h")
            for tt in range(2):
                layernorm_tt(src, out, tt)
            return out

        def transpose2(src, pool=t4):
            """src [128,2,D] bf16 (tokens, dims) -> [128,KD,CH] (dims, toks)."""
            out = pool.tile([128, KD, CH], BF, tag="t4" if pool is t4 else "xft")
            for d in range(KD):
                for tt in range(2):
                    ps = mmp.tile([128, 1024], BF, tag="mm")
                    nc.tensor.transpose(ps[:, 0:128], src[:, tt, ds(d * 128, 128)],
                                        ident[:])
                    nc.vector.tensor_copy(out[:, d, ds(tt * 128, 128)],
                                          ps[:, 0:128])
            return out

        nxt_wqkvo = nxt_w1h = nxt_w2h = None
        h_cur = layernorm(xt)
        for l in range(layers):
            # ---- prefetch next layer's weights + one Wlm slice ----
            if l + 1 < layers:
                nxt_wqkvo = load_wqkvo(l + 1)
                nxt_w1h = load_w1h(l + 1, 0)
                nxt_w2h = load_w2h(l + 1, 0)
            if l < KD - 1:
                nc.gpsimd.dma_start(wlm_sb[:, l + 1, :], wlm[:, l + 1, :])

            # ---- LN1 (already computed) + transpose ----
            hT = transpose2(h_cur)

            # ---- gather hT itself (smallest possible payload); K/V for
            # all four chunks are recomputed locally after the gather, which
            # also re-warms the PE clock before attention ----
            ht_in = dram.tile([XF], BF, tag="htin")
            nc.sync.dma_start(
                ht_in[:].rearrange("(p a b) -> p a b", p=128, a=KD), hT[:])
            ht_out = dram.tile([4, XF], BF, tag="htout")
            nc.gpsimd.collective_compute(
                "AllGather", OP.bypass,
                replica_groups=[[0, 1, 2, 3], [4, 5, 6, 7]],
                ins=[ht_in[:].opt()], outs=[ht_out[:].opt()])

            # ---- q projection overlaps the collective ----
            qT = t4.tile([128, KD, CH], BF, tag="t4")
            for d in range(KD):
                ps = mmp.tile([128, 512], FP, tag="mm")
                for k in range(KD):
                    nc.tensor.matmul(ps[:, :CH], cur_wqkvo[:, k, 0, ds(d * 128, 128)],
                                     hT[:, k, :], start=(k == 0),
                                     stop=(k == KD - 1))
                if with_bias:
                    nc.vector.tensor_scalar_add(qT[:, d, :], ps[:, :CH],
                                                bqk_sb[:, l, 0, d:d + 1])
                else:
                    nc.vector.tensor_copy(qT[:, d, :], ps[:, :CH])
            # dense discarded matmul trains: whenever the PE would idle out
            # the HAM busy-window (collective waits, ACT-bound stretches) we
            # feed it junk matmuls so subsequent real ones run at full clock
            def dummy_train(n):
                for dd in range(n):
                    ps = mmp.tile([128, 512], FP, tag="mm")
                    for k in range(KD):
                        nc.tensor.matmul(ps[:, :CH], cur_wqkvo[:, k, 0, 0:128],
                                         hT[:, k, :], start=(k == 0),
                                         stop=(k == KD - 1))
            dummy_train(22)

            hTall = kvall.tile([128, KD, 4, CH], BF, tag="htall")
            for c in range(4):
                (nc.sync if c % 2 == 0 else nc.scalar).dma_start(
                    hTall[:, :, c, :],
                    ht_out[c].rearrange("(p a b) -> p a b", p=128, a=KD))

            # ---- K (transposed) and V for all four chunks ----
            kTall = kvall.tile([128, KD, 4, CH], BF, tag="ktall")
            vall = kvall.tile([128, 4, 2, H, 65], BF, tag="vall")
            nc.vector.memset(vall[:, :, :, :, 64:65], 1.0)
            for cp in range(2):
                for d in range(KD):
                    ps = mmp.tile([128, 512], FP, tag="mm")
                    for k in range(KD):
                        nc.tensor.matmul(
                            ps[:], cur_wqkvo[:, k, 1, ds(d * 128, 128)],
                            hTall[:, k, ds(cp * 2, 2), :], start=(k == 0),
                            stop=(k == KD - 1))
                    if with_bias:
                        nc.vector.tensor_scalar_add(
                            kTall[:, d, ds(cp * 2, 2), :],
                            ps[:].rearrange("p (c b) -> p c b", c=2),
                            bqk_sb[:, l, 1, d:d + 1])
                    else:
                        nc.vector.tensor_copy(
                            kTall[:, d, ds(cp * 2, 2), :],
                            ps[:].rearrange("p (c b) -> p c b", c=2))
                for ci in range(2):
                    c = cp * 2 + ci
                    for tt in range(2):
                        ps = mmp.tile([128, 512], FP, tag="mm")
                        for k in range(KD):
                            nc.tensor.matmul(
                                ps[:, :D], hTall[:, k, c, ds(tt * 128, 128)],
                                cur_wqkvo[:, k, 2, :], start=(k == 0),
                                stop=(k == KD - 1))
                        nc.vector.tensor_copy(
                            vall[:, c, tt, :, 0:64],
                            ps[:].rearrange("p (h e) -> p h e", h=H))

            # ---- attention: per-head PSUM accumulation over the 4 chunks.
            # attT is assembled with RAW (unnormalized) values; denominators
            # collect in dram, then one compact reciprocal + a broadcast DMA
            # into transposed layout + a single in-place multiply normalize.
            attT = t4.tile([128, KD, CH], BF, tag="t4")
            # araw rows 0-63 = numerators; row 64 = denominators
            araw = attp.tile([128, H, CH], BF, tag="araw", name=f"araw{l}")
            rbcT = rbcp.tile([128, KD, CH], BF, tag="rbcT", name=f"rbcT{l}")
            wops = [mop.tile([128, D], FP, tag="mo", name=f"wo{l}_{t}")
                    for t in range(2)]
            normed = set()

            def norm_dt(dt2):
                """normalize a dt column, then immediately fold its Wo
                contribution in - real PE work amid the attention stream.
                The denominator broadcast is two tiny PE matmuls (ones-row
                outer product) instead of a DMA round trip."""
                if dt2 in normed or dt2 >= KD:
                    return
                normed.add(dt2)
                bc = mmp.tile([128, 512], FP, tag="mm")
                for pa in range(2):
                    nc.tensor.matmul(bc[pa * 64:(pa + 1) * 64, 0:CH],
                                     ones_sb[64:65, :],
                                     araw[64:65, 2 * dt2 + pa, :],
                                     start=True, stop=True)
                with nc.allow_low_precision(reason="softmax denom recip bf16"):
                    nc.vector.reciprocal(rbcT[:, dt2, :], bc[:, 0:CH])
                nc.vector.tensor_tensor(attT[:, dt2, :], attT[:, dt2, :],
                                        rbcT[:, dt2, :], OP.mult)
                for tt in range(2):
                    nc.tensor.matmul(wops[tt][:],
                                     attT[:, dt2, ds(tt * 128, 128)],
                                     cur_wqkvo[:, dt2, 3, :],
                                     start=(dt2 == 0), stop=(dt2 == KD - 1))

            for hp in range(H // 2):
                avt = {}
                for hh in (2 * hp, 2 * hp + 1):
                    avt[hh] = avp.tile([65, CH], FP, tag="av", name=f"av{l}_{hh}")
                for c in range(4):
                    ext = {}
                    for hh in (2 * hp, 2 * hp + 1):
                        pb = (hh % 2) * 64
                        sps = spp.tile([128, 2, CH], FP, tag="sp")
                        for kt in range(2):
                            nc.tensor.matmul(
                                sps[:, kt, :],
                                kTall[pb:pb + 64, hp, c, ds(kt * 128, 128)],
                                qT[pb:pb + 64, hp, :],
                                start=True, stop=True)
                        ex = expp.tile([128, 2, CH], BF, tag="exp")
                        nc.scalar.activation(ex[:], sps[:], AF.Exp)
                        # one of four mask-mults rides on gpsimd
                        eng = nc.gpsimd if (c == 3 and hh % 2 == 0) else nc.vector
                        eng.tensor_tensor(ex[:], ex[:], msk_sb[:, c, :, :],
                                          OP.mult)
                        ext[hh] = ex
                    for hh in (2 * hp, 2 * hp + 1):
                        for kt in range(2):
                            nc.tensor.matmul(
                                avt[hh][:], vall[:, c, kt, hh, :],
                                ext[hh][:, kt, :],
                                start=(c == 0 and kt == 0),
                                stop=(c == 3 and kt == 1))
                for hh in (2 * hp, 2 * hp + 1):
                    pb = (hh % 2) * 64
                    avps = avt[hh]
                    # eagerly drain PSUM so later heads are never blocked
                    if pb == 0:
                        nc.vector.tensor_copy(attT[0:64, hp, :], avps[0:64, :])
                        nc.vector.tensor_copy(araw[64:65, hh, :], avps[64:65, :])
                    else:
                        nc.vector.tensor_copy(araw[0:65, hh, :], avps[0:65, :])
                        nc.scalar.dma_start(attT[64:128, hp, :], araw[0:64, hh, :])
                # normalize a dt-column one pair after its denominators landed
                if hp >= 1:
                    norm_dt(hp - 1)
            norm_dt(3)
            dummy_train(3)

            # ---- Wo accumulated during attention: bias + residual + LN2 ----
            h2 = hpool.tile([128, 2, D], BF, tag="h")
            if with_bias:
                bo_b = bcp.tile([128, D], FP, tag="bc")
                bo_src = bo2[l, 0]
                nc.sync.dma_start(bo_b[:], bass.AP(
                    tensor=bo_src.tensor, offset=bo_src.offset,
                    ap=[[0, 128]] + list(bo_src.ap)))
            for tt in range(2):
                if with_bias:
                    nc.vector.tensor_tensor(wops[tt][:], wops[tt][:], bo_b[:],
                                            OP.add)
                nc.vector.tensor_tensor(xt[:, tt, :], xt[:, tt, :],
                                        wops[tt][:], OP.add)
                layernorm_tt(xt, h2, tt)

            # ---- LN2 + transpose ----
            h2T = transpose2(h2)

            # ---- MLP ----
            if with_bias:
                b2_b = bcp.tile([128, D], FP, tag="bc")
                b2_src = bo2[l, 1]
                nc.sync.dma_start(b2_b[:], bass.AP(
                    tensor=b2_src.tensor, offset=b2_src.offset,
                    ap=[[0, 128]] + list(b2_src.ap)))
            x2ps = [mop.tile([128, D], FP, tag="mo", name=f"mo{l}_{kk}")
                    for kk in range(2)]
            for mp in range(MD // 2):
                gps = mmp.tile([128, 2, CH], FP, tag="mm")
                for mi in range(2):
                    m = mp * 2 + mi
                    w1h = cur_w1[m // (MD // 2)]
                    mm = m % (MD // 2)
                    for k in range(KD):
                        nc.tensor.matmul(gps[:, mi, :],
                                         w1h[:, k, ds(mm * 128, 128)],
                                         h2T[:, k, :], start=(k == 0),
                                         stop=(k == KD - 1))
                gt = gtp.tile([128, 2, CH], BF, tag="gt")
                if with_bias:
                    for mi in range(2):
                        m = mp * 2 + mi
                        nc.scalar.activation(gt[:, mi, :], gps[:, mi, :], AF.Gelu,
                                             bias=b1_sb[:, l, m:m + 1])
                else:
                    nc.scalar.activation(gt[:], gps[:], AF.Gelu)
                for mi in range(2):
                    m = mp * 2 + mi
                    w2h = cur_w2[m // (MD // 2)]
                    for tt in range(2):
                        nc.tensor.matmul(x2ps[tt][:], gt[:, mi, ds(tt * 128, 128)],
                                         w2h[:, m % (MD // 2), :],
                                         start=(m == 0), stop=(m == MD - 1))
                if mp == 3 and l + 1 < layers:
                    # the current layer's first halves had their last read at
                    # m=7 (this iteration) - their ring slots are now free
                    nxt_w1 = [nxt_w1h, load_w1h(l + 1, 1)]
                    nxt_w2 = [nxt_w2h, load_w2h(l + 1, 1)]
            h_nxt = hpool.tile([128, 2, D], BF, tag="h")
            for tt in range(2):
                if with_bias:
                    nc.vector.tensor_tensor(x2ps[tt][:], x2ps[tt][:], b2_b[:],
                                            OP.add)
                nc.vector.tensor_tensor(xt[:, tt, :], xt[:, tt, :], x2ps[tt][:],
                                        OP.add)
                layernorm_tt(xt, h_nxt, tt)
            h_cur = h_nxt
            if l + 1 < layers:
                cur_wqkvo, cur_w1, cur_w2 = nxt_wqkvo, nxt_w1, nxt_w2

        # ---- final LN; AllGather hidden state across all 8 (Shared/RDH) ----
        xfT = transpose2(h_cur, pool=xftp)
        xf_in = dram.tile([XF], BF, tag="xfin")
        nc.sync.dma_start(
            xf_in[:].rearrange("(p a b) -> p a b", p=128, a=KD), xfT[:])
        xf_out = dram.tile([NC, XF], BF, tag="xfout", addr_space="Shared")
        nc.gpsimd.collective_compute(
            "AllGather", OP.bypass,
            replica_groups=[list(range(NC))],
            ins=[xf_in[:].opt()], outs=[xf_out[:].opt()])

        hp_pools = ((mmp, "mm"), (mop, "mo"), (spp, "sp"))
        dma_engs = (nc.sync, nc.scalar, nc.gpsimd)
        gi = 0
        LGB = [0, 5, 9, 13]  # logits go out in chunks of 5/4/4 vocab tiles

        def head_group(get_lhs, tg):
            """One 128-token group: all NT vocab tiles -> three chunked DMAs.
            tg None => warmup-only (compute and discard)."""
            nonlocal gi
            lgt = None
            cj = 0
            for n in range(NT):
                if tg is not None and n == LGB[cj]:
                    lgt = lgp.tile([128, 5, 512], BF, tag="lg",
                                   name=f"lg{tg}_{cj}")
                nsz = min(512, VS - n * 512)
                pool_i, ptag = hp_pools[gi % 3]
                ps = pool_i.tile([128, 512], FP, tag=ptag,
                                 name=f"hps{tg}_{n}")
                for k in range(KD):
                    nc.tensor.matmul(
                        ps[:, :nsz], get_lhs(k),
                        wlm_sb[:, k, ds(n * 512, nsz)],
                        start=(k == 0), stop=(k == KD - 1))
                gi += 1
                if tg is None:
                    continue
                nb = n - LGB[cj]
                if gi % 2 == 0:
                    nc.vector.tensor_copy(lgt[:, nb, :nsz], ps[:, :nsz])
                else:
                    nc.scalar.activation(lgt[:, nb, :nsz], ps[:, :nsz], AF.Copy)
                if n == LGB[cj + 1] - 1:
                    nw = LGB[cj + 1] - LGB[cj]
                    dma_engs[(tg + cj) % 3].dma_start(
                        logits[tg, :, ds(LGB[cj] * 512, nw * 512)],
                        lgt[:, 0:nw, :])
                    cj += 1

        # pass 1: warmup on local tokens while the AllGather flies (discard)
        for mt in range(2):
            head_group(lambda k, mt=mt: xfT[:, k, ds(mt * 128, 128)], None)

        # read the gathered slots back pairwise (ring of 2 tiles) so the
        # next pair's DMAs overlap the current pair's matmuls
        for p in range(4):
            xa = xfap.tile([128, KD, 2, CH], BF, tag="xfa", name=f"xfa{p}")
            for ci in range(2):
                dma_engs[(p * 2 + ci) % 3].dma_start(
                    xa[:, :, ci, :],
                    xf_out[p * 2 + ci, :].rearrange("(p a b) -> p a b",
                                                    p=128, a=KD))
            for ci in range(2):
                c = p * 2 + ci
                for mt in range(2):
                    head_group(
                        lambda k, xa=xa, ci=ci, mt=mt:
                            xa[:, k, ci, ds(mt * 128, 128)],
                        c * 2 + mt)

    nc.compile()
    return nc


_CACHE = {}


def _get_program(with_bias=True):
    key = ("nc", with_bias)
    if key not in _CACHE:
        _CACHE[key] = build_program(with_bias=with_bias)
    return _CACHE[key]


def _to_pmaj(w):
    """[.., D_in, N] -> [.., 128, D_in//128, N] partition-major."""
    shp = w.shape
    kd = shp[-2] // 128
    return np.ascontiguousarray(
        w.reshape(*shp[:-2], kd, 128, shp[-1]).swapaxes(-3, -2))


def _prep_inputs(inputs):
    f = lambda k: np.asarray(inputs[k], np.float32)
    bf = ml_dtypes.bfloat16
    idx = np.asarray(inputs["idx"]).astype(np.int64)
    tok_emb, pos_emb = f("tok_emb"), f("pos_emb")
    x0 = tok_emb[idx] + pos_emb[None, :T]          # [B, T, D]
    x0 = x0.reshape(NC, 2, 128, D).transpose(0, 2, 1, 3).copy()  # [NC,128,2,D]

    ln1_g, ln1_b = f("ln1_g"), f("ln1_b")
    ln2_g, ln2_b = f("ln2_g"), f("ln2_b")
    Wq, bq = f("Wq"), f("bq")
    Wk, bk = f("Wk"), f("bk")
    Wv, bv = f("Wv"), f("bv")
    Wo, bo = f("Wo"), f("bo")
    W1, b1 = f("W1"), f("b1")
    W2, b2 = f("W2"), f("b2")
    lnf_g, lnf_b = f("lnf_g"), f("lnf_b")
    Wlm, blm = f("Wlm"), f("blm")

    sc = 1.0 / np.sqrt(HS)
    wqe = ln1_g[:, :, None] * Wq * sc
    bqe = (np.einsum("ld,ldm->lm", ln1_b, Wq) + bq) * sc
    wke = ln1_g[:, :, None] * Wk
    bke = np.einsum("ld,ldm->lm", ln1_b, Wk) + bk
    wve = ln1_g[:, :, None] * Wv
    bve = np.einsum("ld,ldm->lm", ln1_b, Wv) + bv
    boe = np.einsum("lm,lmd->ld", bve, Wo) + bo
    w1e = ln2_g[:, :, None] * W1
    b1e = np.einsum("ld,ldf->lf", ln2_b, W1) + b1
    wlme = lnf_g[:, None] * Wlm
    blme = lnf_b @ Wlm + blm

    bqk = np.stack([bqe, bke], axis=1)             # [L, 2, D]
    bqk = bqk.reshape(L, 2, KD, 128).transpose(3, 0, 1, 2).copy()
    b1t = b1e.reshape(L, MD, 128).transpose(2, 0, 1).copy()
    bo2 = np.stack([boe, b2], axis=1)              # [L, 2, D]

    wlmp = np.zeros((D, NC * VS), np.float32)
    wlmp[:, :V] = wlme

    masks = []
    for core in range(NC):
        cc = core % 4
        qpos = cc * CH + np.arange(CH)
        m = np.empty((128, 4, 2, CH), np.float32)
        for kc in range(4):
            for kt in range(2):
                kpos = kc * CH + kt * 128 + np.arange(128)
                m[:, kc, kt, :] = (kpos[:, None] <= qpos[None, :]).astype(np.float32)
        masks.append(m.astype(bf))

    shared = dict(wq=_to_pmaj(wqe.astype(bf)), wk=_to_pmaj(wke.astype(bf)),
                  wv=_to_pmaj(wve.astype(bf)), wo=_to_pmaj(Wo.astype(bf)),
                  w1=_to_pmaj(w1e.astype(bf)), w2=_to_pmaj(W2.astype(bf)),
                  bqk=bqk, b1t=b1t, bo2=np.ascontiguousarray(bo2))
    in_maps = []
    for core in range(NC):
        m = dict(shared)
        m["x0"] = np.ascontiguousarray(x0[core])
        m["msk"] = masks[core]
        m["wlm"] = _to_pmaj(wlmp[:, core * VS:(core + 1) * VS].astype(bf))
        in_maps.append(m)
    return in_maps, blme


def _run(inputs, trace=False):
    in_maps, blme = _prep_inputs(inputs)
    with_bias = bool(np.any(in_maps[0]["bo2"]))
    nc = _get_program(with_bias=with_bias)
    res = bass_utils.run_bass_kernel_spmd(nc, in_maps, core_ids=list(range(NC)),
                                          trace=trace)
    lg = np.concatenate(
        [np.asarray(res.results[c]["logits"]).astype(np.float32)
         .reshape(BT, NTP)[:, :VS] for c in range(NC)], axis=1)
    out = lg[:, :V]
    if np.any(blme):
        out = out + blme[None, :]
    return out.reshape(B, T, V).astype(np.float32), res


def kernel(**inputs) -> np.ndarray:
    out, _ = _run(inputs, trace=False)
    return out
